# revision 29
# baseline (speedup 1.0000x reference)
"""Multi-head attention forward on 8 Trainium2 NeuronCores.

Problem (hardcoded): B=2, S=2048, H=1024, NH=16 heads, HD=64.
  out  = softmax((x Wq^T + bq)(x Wk^T + bk)^T / sqrt(HD)) (x Wv^T + bv) Wo^T + bo
  avg  = attn.mean(heads).mean(queries)           # [B, S]

Sharding: core c = 4*b + g handles batch b and head group g (4 heads,
256 channels).  Data parallel over B, tensor parallel over heads.

Per-core algorithm (all layouts chosen so nothing is ever transposed on
device):
  - xT [H, S] (host pre-transposed, bf16)
  - QT/KT [256, S] channel-major via matmul(lhsT=WqT, rhs=xT).
    * softmax scale folded into Wq/bq on host
    * bq added during PSUM->SBUF copy (ACT per-partition bias)
    * bk dropped entirely: (x Wq^T + bq)(bk)^T is constant along the key
      axis, so it cancels in softmax
    * bv folded into the host-side output bias: bo_eff = bo + bv @ Wo^T
  - V [S, 4*65] row-major with a ones column per head (column 64):
    the ones column makes the attn@V matmul also produce the softmax
    denominators for free.
  - main loop (head pair p, query quarter qq, key tile kt):
      scoresT[k, q] psum  = two row-packed K=64 matmuls (heads of pair)
      expT = ACT Exp (scores are O(1) so no max subtraction needed;
             single pass over all 16.8M score elements)
      ctxT_u[d, q] psum  += matmul(lhsT=[V_h | 1], rhs=expT_h)  (M=65)
      row 64 of ctxT_u = denom[q]
  - r = 1/denom -> replicate across partitions (DRAM bounce broadcast)
  - ctxT = ctxT_u * r  (normalization applied here, per head)
  - avg_u[k] += sum_q expT[k, q] * r[q]  (DVE tensor_tensor_reduce)
  - out_u[s, o] = matmul(lhsT=ctxT, rhs=WoT); DMA PSUM->DRAM
  - host: out[b] = sum_g out_part + bo_eff; avg[b] = sum_g avg_u/(NH*S)
"""

import numpy as np
import ml_dtypes
from contextlib import ExitStack

import concourse.bass as bass
import concourse.mybir as mybir
import concourse.tile as tile
from concourse import bacc
from concourse.bass_utils import run_bass_kernel_spmd

B, S, H = 2, 2048, 1024
NH, HD = 16, 64
NCORES = 8
GROUPS = 4          # head groups across cores
HPG = NH // GROUPS  # 4 heads per core
OC = HPG * HD       # 256 output channels per core
PAIRS = HPG // 2    # 2 head pairs
QQ = 4              # query quarters
QW = S // QQ        # 512
NKT = S // 128      # 16 key tiles
P = 128

F32 = mybir.dt.float32
BF16 = mybir.dt.bfloat16
NP_BF16 = ml_dtypes.bfloat16

# Set False to run the whole datapath in fp32 (slower, ~1e-6 error).
USE_BF16 = True
DT = BF16 if USE_BF16 else F32
NPDT = NP_BF16 if USE_BF16 else np.float32

_CACHE = {}


def _trace_kernel():
    nc = bacc.Bacc(
        "TRN2", target_bir_lowering=False, debug=False, num_devices=NCORES
    )

    xT_d = nc.dram_tensor("xT", [H, S], DT, kind="ExternalInput").ap()
    wqT_d = nc.dram_tensor("wqT", [H, OC], DT, kind="ExternalInput").ap()
    wkT_d = nc.dram_tensor("wkT", [H, OC], DT, kind="ExternalInput").ap()
    wvT_d = nc.dram_tensor("wvT", [H, OC], DT, kind="ExternalInput").ap()
    woT_d = nc.dram_tensor("woT", [OC, H], DT, kind="ExternalInput").ap()
    bq_d = nc.dram_tensor("bq", [OC], F32, kind="ExternalInput").ap()
    out_d = nc.dram_tensor("out_part", [S, H], F32, kind="ExternalOutput").ap()
    avg_d = nc.dram_tensor("avg_part", [P, NKT], F32, kind="ExternalOutput").ap()

    Copy = mybir.ActivationFunctionType.Copy
    Ident = mybir.ActivationFunctionType.Identity
    Exp = mybir.ActivationFunctionType.Exp
    MUL = mybir.AluOpType.mult
    ADD = mybir.AluOpType.add

    with tile.TileContext(nc) as tc, ExitStack() as ctx:
        persist = ctx.enter_context(tc.tile_pool(name="persist", bufs=1))

        # ---- load inputs (weights first so matmuls start early) ---------
        bq_sb = persist.tile([P, OC // P], F32)
        nc.sync.dma_start(bq_sb[:], bq_d.rearrange("(m p) -> p m", p=P))
        wq_sb = persist.tile([P, H // P, OC], DT)
        nc.sync.dma_start(wq_sb[:], wqT_d.rearrange("(t p) c -> p t c", p=P))
        wk_sb = persist.tile([P, H // P, OC], DT)
        nc.sync.dma_start(wk_sb[:], wkT_d.rearrange("(t p) c -> p t c", p=P))
        wv_sb = persist.tile([P, H // P, OC], DT)
        nc.sync.dma_start(wv_sb[:], wvT_d.rearrange("(t p) c -> p t c", p=P))
        wo_sb = persist.tile([P, OC // P, H], DT)         # [128, 2, 1024]
        nc.sync.dma_start(wo_sb[:], woT_d.rearrange("(t p) o -> p t o", p=P))
        xT_sb = persist.tile([P, H // P, S], DT)          # [128, 8, 2048]
        xT_r = xT_d.rearrange("(t p) s -> p t s", p=P)
        for kt in range(H // P):
            nc.sync.dma_start(xT_sb[:, kt:kt + 1, :], xT_r[:, kt:kt + 1, :])

        QT_sb = persist.tile([P, OC // P, S], DT)         # [128, 2, 2048]
        KT_sb = persist.tile([P, OC // P, S], DT)
        V_sb = persist.tile([P, NKT, HPG * (HD + 1)], DT)  # [128, 16, 260]
        ctxT_sb = persist.tile([P, OC // P, S], DT)
        avg_slots = persist.tile([P, NKT, PAIRS * QQ], F32)
        avg_sb = persist.tile([P, NKT], F32)

        # ---- phase 1: Q/K/V projections ---------------------------------
        with tc.tile_pool(name="p1psum", bufs=2, space="PSUM") as p1:
            for which, w_sb, dst, bias in (
                ("q", wq_sb, QT_sb, True),
                ("k", wk_sb, KT_sb, False),
            ):
                for m in range(OC // P):
                    for nh in range(2):  # halves of S
                        ps = p1.tile([P, S // 2], F32, tag="psqk")
                        for kt in range(H // P):
                            for ncj in range(2):
                                nc.tensor.matmul(
                                    ps[:, ncj * 512:(ncj + 1) * 512],
                                    lhsT=w_sb[:, kt, m * P:(m + 1) * P],
                                    rhs=xT_sb[:, kt,
                                              nh * 1024 + ncj * 512:
                                              nh * 1024 + (ncj + 1) * 512],
                                    start=(kt == 0),
                                    stop=(kt == H // P - 1),
                                )
                        if bias:
                            nc.scalar.activation(
                                dst[:, m, nh * 1024:(nh + 1) * 1024],
                                ps[:],
                                Ident,
                                bias=bq_sb[:, m:m + 1],
                            )
                        else:
                            nc.scalar.activation(
                                dst[:, m, nh * 1024:(nh + 1) * 1024],
                                ps[:],
                                Copy,
                            )
            # V projection is interleaved into the first attention sweep;
            # only its ones columns are prefilled here (disjoint from the
            # per-head 64-wide slices the copies write).
            v4 = V_sb[:].rearrange("p t (g c) -> p t g c", c=HD + 1)
            nc.vector.memset(v4[:, :, :, HD:HD + 1], 1.0)

        # ---- phase 2: attention main loop -------------------------------
        ones_f32 = persist.tile([1, P], F32)
        nc.vector.memset(ones_f32[:], 1.0)

        p2stack = ExitStack()
        with tc.tile_pool(name="expp", bufs=2) as expp, \
             tc.tile_pool(name="rp", bufs=2) as rp, \
             tc.tile_pool(name="scr", bufs=4) as scr:
            p2 = p2stack.enter_context(
                tc.tile_pool(name="p2psum", bufs=2, space="PSUM"))
            p2c = p2stack.enter_context(
                tc.tile_pool(name="p2ctx", bufs=2, space="PSUM"))
            # psv lives only during sweep 0; p2r opens after it closes so
            # both fit in the remaining two PSUM banks
            psv_stack = ExitStack()
            psv_pool = psv_stack.enter_context(
                tc.tile_pool(name="psv", bufs=2, space="PSUM"))
            p2r = None
            for pair in range(PAIRS):
                for qq in range(QQ):
                    q0 = qq * QW
                    expT = expp.tile([P, NKT, 2 * QW], DT, tag="expT")
                    cps = [
                        p2c.tile([HD + 1, QW], F32, tag="cps", name=f"cps{i}")
                        for i in range(2)
                    ]

                    def vmm(kt):
                        for hl in range(2):
                            h = 2 * pair + hl
                            nc.tensor.matmul(
                                cps[hl][:],
                                lhsT=V_sb[:, kt,
                                          h * (HD + 1):(h + 1) * (HD + 1)],
                                rhs=expT[:, kt, hl * QW:(hl + 1) * QW],
                                start=(kt == 0),
                                stop=(kt == NKT - 1),
                            )

                    # scores+exp run one kt ahead of the attn@V matmuls so
                    # PE never waits on the exp of the tile it consumes.
                    # Sweep 0 additionally emits the V projection for key
                    # tile kt just before using it one iteration later.
                    for kt in range(NKT):
                        if pair == 0 and qq == 0:
                            psv = psv_pool.tile([P, OC], F32, tag="psv")
                            for hkt in range(H // P):
                                nc.tensor.matmul(
                                    psv[:],
                                    lhsT=xT_sb[:, hkt, kt * P:(kt + 1) * P],
                                    rhs=wv_sb[:, hkt, :],
                                    start=(hkt == 0),
                                    stop=(hkt == H // P - 1),
                                )
                            nc.vector.tensor_copy(
                                v4[:, kt, :, 0:HD],
                                psv[:].rearrange("p (g c) -> p g c", c=HD),
                            )
                        ps_s = p2.tile([P, 2 * QW], F32, tag="ps_s")
                        for hl in range(2):
                            lo = 64 * hl
                            nc.tensor.matmul(
                                ps_s[:, hl * QW:(hl + 1) * QW],
                                lhsT=KT_sb[lo:lo + 64, pair,
                                           kt * P:(kt + 1) * P],
                                rhs=QT_sb[lo:lo + 64, pair, q0:q0 + QW],
                                start=True,
                                stop=True,
                            )
                        nc.scalar.activation(expT[:, kt, :], ps_s[:], Exp)
                        if kt > 0:
                            vmm(kt - 1)
                    vmm(NKT - 1)
                    if pair == 0 and qq == 0:
                        psv_stack.close()
                        p2r = p2stack.enter_context(
                            tc.tile_pool(name="p2rr", bufs=2, space="PSUM"))

                    # denominators -> r -> replicate across partitions via a
                    # K=1 PE broadcast matmul (cheap, no DMA round trip).
                    # Both heads' chains interleaved so they overlap.
                    rrep = rp.tile([P, 2 * QW], DT, tag="rrep")
                    r32s, rpss = [], []
                    for hl in range(2):
                        r32 = scr.tile([1, QW], F32, tag="r32",
                                       name=f"r32_{hl}")
                        nc.vector.reciprocal(r32[:], cps[hl][HD:HD + 1, :])
                        r32s.append(r32)
                    for hl in range(2):
                        rps = p2r.tile([P, QW], F32, tag="rps",
                                       name=f"rps_{hl}")
                        nc.tensor.matmul(rps[:], lhsT=ones_f32[:],
                                         rhs=r32s[hl][:], start=True,
                                         stop=True)
                        rpss.append(rps)
                    for hl in range(2):
                        nc.scalar.copy(
                            rrep[:, hl * QW:(hl + 1) * QW], rpss[hl][:])
                    for hl in range(2):
                        # normalized ctx (bf16) while copying out of psum
                        nc.vector.tensor_tensor(
                            ctxT_sb[64 * hl:64 * hl + 64, pair, q0:q0 + QW],
                            cps[hl][0:HD, :],
                            rrep[64 * hl:64 * hl + 64,
                                 hl * QW:(hl + 1) * QW],
                            MUL,
                        )

                    # avg_attention partial sums: the final sweep's batch is
                    # deferred past the output projection so it overlaps
                    slot = pair * QQ + qq
                    if not (pair == PAIRS - 1 and qq == QQ - 1):
                        for kt in range(NKT):
                            tt_out = scr.tile([P, 2 * QW], DT, tag="tt_out")
                            nc.vector.affine_mul_reduce(
                                out=tt_out[:],
                                accum_out=avg_slots[:, kt, slot:slot + 1],
                                in0=expT[:, kt, :],
                                in1=rrep[:],
                                scale=1.0,
                                bias=0.0,
                            )
                    else:
                        last_expT, last_rrep, last_slot = expT, rrep, slot

            # ---- phase 3: output projection -----------------------------
            # psum->sbuf copies go on ScalarE so they don't queue behind
            # the remaining avg reductions on DVE
            p2stack.close()  # release phase-2 psum pools
            with tc.tile_pool(name="p3psum", bufs=2, space="PSUM") as p3, \
                 tc.tile_pool(name="p3out", bufs=2) as p3o:
                for st in range(NKT):
                    po = p3.tile([P, H], F32, tag="po")
                    for ot in range(OC // P):
                        for ncj in range(2):
                            nc.tensor.matmul(
                                po[:, ncj * 512:(ncj + 1) * 512],
                                lhsT=ctxT_sb[:, ot, st * P:(st + 1) * P],
                                rhs=wo_sb[:, ot, ncj * 512:(ncj + 1) * 512],
                                start=(ot == 0),
                                stop=(ot == OC // P - 1),
                            )
                    ost = p3o.tile([P, H], F32, tag="ost")
                    nc.scalar.copy(ost[:], po[:])
                    nc.sync.dma_start(out_d[st * P:(st + 1) * P, :], ost[:])
                    tt_out = scr.tile([P, 2 * QW], DT, tag="tt_out")
                    nc.vector.affine_mul_reduce(
                        out=tt_out[:],
                        accum_out=avg_slots[:, st, last_slot:last_slot + 1],
                        in0=last_expT[:, st, :],
                        in1=last_rrep[:],
                        scale=1.0,
                        bias=0.0,
                    )

        nc.vector.tensor_reduce(
            avg_sb[:], avg_slots[:], axis=mybir.AxisListType.X, op=ADD
        )
        nc.sync.dma_start(avg_d[:, :], avg_sb[:])

    nc.compile()
    return nc


def _get_nc():
    if "nc" not in _CACHE:
        _CACHE["nc"] = _trace_kernel()
    return _CACHE["nc"]


def _prep_in_maps(x, Wq, Wk, Wv, Wo, bq, bk, bv, bo):
    scale = HD ** -0.5
    in_maps = []
    for c in range(NCORES):
        b, g = divmod(c, GROUPS)
        sl = slice(g * OC, (g + 1) * OC)
        in_maps.append({
            "xT": np.ascontiguousarray(x[b].T).astype(NPDT),
            "wqT": np.ascontiguousarray((Wq[sl] * scale).T).astype(NPDT),
            "wkT": np.ascontiguousarray(Wk[sl].T).astype(NPDT),
            "wvT": np.ascontiguousarray(Wv[sl].T).astype(NPDT),
            "woT": np.ascontiguousarray(Wo[:, sl].T).astype(NPDT),
            "bq": (bq[sl] * scale).astype(np.float32),
        })
    return in_maps


def _postprocess(results, Wo, bv, bo):
    bo_eff = (bo + bv @ Wo.T).astype(np.float32)
    out = np.zeros((B, S, H), np.float32)
    avg = np.zeros((B, S), np.float32)
    for c in range(NCORES):
        b = c // GROUPS
        out[b] += results[c]["out_part"]
        avg[b] += results[c]["avg_part"].T.ravel()
    out += bo_eff
    avg /= NH * S
    return out, avg


def kernel(x, Wq, Wk, Wv, Wo, bq, bk, bv, bo, _trace=False):
    x, Wq, Wk, Wv, Wo = (np.asarray(a, np.float32) for a in (x, Wq, Wk, Wv, Wo))
    bq, bk, bv, bo = (np.asarray(a, np.float32) for a in (bq, bk, bv, bo))
    nc = _get_nc()
    in_maps = _prep_in_maps(x, Wq, Wk, Wv, Wo, bq, bk, bv, bo)
    res = run_bass_kernel_spmd(nc, in_maps, list(range(NCORES)), trace=_trace)
    out, avg = _postprocess(res.results, Wo, bv, bo)
    if _trace:
        _CACHE["last_result"] = res
    return out, avg


# revision 30
# speedup vs baseline: 1.0142x; 1.0142x over previous
"""Multi-head attention forward on 8 Trainium2 NeuronCores.

Problem (hardcoded): B=2, S=2048, H=1024, NH=16 heads, HD=64.
  out  = softmax((x Wq^T + bq)(x Wk^T + bk)^T / sqrt(HD)) (x Wv^T + bv) Wo^T + bo
  avg  = attn.mean(heads).mean(queries)           # [B, S]

Sharding: core c = 4*b + g handles batch b and head group g (4 heads,
256 channels).  Data parallel over B, tensor parallel over heads.

Per-core algorithm (all layouts chosen so nothing is ever transposed on
device):
  - xT [H, S] (host pre-transposed, bf16)
  - QT/KT [256, S] channel-major via matmul(lhsT=WqT, rhs=xT).
    * softmax scale folded into Wq/bq on host
    * bq added during PSUM->SBUF copy (ACT per-partition bias)
    * bk dropped entirely: (x Wq^T + bq)(bk)^T is constant along the key
      axis, so it cancels in softmax
    * bv folded into the host-side output bias: bo_eff = bo + bv @ Wo^T
  - V [S, 4*65] row-major with a ones column per head (column 64):
    the ones column makes the attn@V matmul also produce the softmax
    denominators for free.
  - main loop (head pair p, query quarter qq, key tile kt):
      scoresT[k, q] psum  = two row-packed K=64 matmuls (heads of pair)
      expT = ACT Exp (scores are O(1) so no max subtraction needed;
             single pass over all 16.8M score elements)
      ctxT_u[d, q] psum  += matmul(lhsT=[V_h | 1], rhs=expT_h)  (M=65)
      row 64 of ctxT_u = denom[q]
  - r = 1/denom -> replicate across partitions (DRAM bounce broadcast)
  - ctxT = ctxT_u * r  (normalization applied here, per head)
  - avg_u[k] += sum_q expT[k, q] * r[q]  (DVE tensor_tensor_reduce)
  - out_u[s, o] = matmul(lhsT=ctxT, rhs=WoT); DMA PSUM->DRAM
  - host: out[b] = sum_g out_part + bo_eff; avg[b] = sum_g avg_u/(NH*S)
"""

import numpy as np
import ml_dtypes
from contextlib import ExitStack

import concourse.bass as bass
import concourse.mybir as mybir
import concourse.tile as tile
from concourse import bacc
from concourse.bass_utils import run_bass_kernel_spmd

B, S, H = 2, 2048, 1024
NH, HD = 16, 64
NCORES = 8
GROUPS = 4          # head groups across cores
HPG = NH // GROUPS  # 4 heads per core
OC = HPG * HD       # 256 output channels per core
PAIRS = HPG // 2    # 2 head pairs
QQ = 4              # query quarters
QW = S // QQ        # 512
NKT = S // 128      # 16 key tiles
P = 128

F32 = mybir.dt.float32
BF16 = mybir.dt.bfloat16
NP_BF16 = ml_dtypes.bfloat16

# Set False to run the whole datapath in fp32 (slower, ~1e-6 error).
USE_BF16 = True
DT = BF16 if USE_BF16 else F32
NPDT = NP_BF16 if USE_BF16 else np.float32

_CACHE = {}


def _trace_kernel():
    nc = bacc.Bacc(
        "TRN2", target_bir_lowering=False, debug=False, num_devices=NCORES
    )

    xT_d = nc.dram_tensor("xT", [H, S], DT, kind="ExternalInput").ap()
    wqT_d = nc.dram_tensor("wqT", [H, OC], DT, kind="ExternalInput").ap()
    wkT_d = nc.dram_tensor("wkT", [H, OC], DT, kind="ExternalInput").ap()
    wvT_d = nc.dram_tensor("wvT", [H, OC], DT, kind="ExternalInput").ap()
    woT_d = nc.dram_tensor("woT", [OC, H], DT, kind="ExternalInput").ap()
    bq_d = nc.dram_tensor("bq", [OC], F32, kind="ExternalInput").ap()
    out_d = nc.dram_tensor("out_part", [S, H], F32, kind="ExternalOutput").ap()
    avg_d = nc.dram_tensor("avg_part", [P, NKT], F32, kind="ExternalOutput").ap()

    Copy = mybir.ActivationFunctionType.Copy
    Ident = mybir.ActivationFunctionType.Identity
    Exp = mybir.ActivationFunctionType.Exp
    MUL = mybir.AluOpType.mult
    ADD = mybir.AluOpType.add

    with tile.TileContext(nc) as tc, ExitStack() as ctx:
        persist = ctx.enter_context(tc.tile_pool(name="persist", bufs=1))

        # ---- load inputs: the first Q matmul needs bq+wq+xT[kt=0], so
        # those DMAs go first; late-use weights (wv, wo) follow the x rows
        bq_sb = persist.tile([P, OC // P], F32)
        nc.sync.dma_start(bq_sb[:], bq_d.rearrange("(m p) -> p m", p=P))
        wq_sb = persist.tile([P, H // P, OC], DT)
        nc.sync.dma_start(wq_sb[:], wqT_d.rearrange("(t p) c -> p t c", p=P))
        xT_sb = persist.tile([P, H // P, S], DT)          # [128, 8, 2048]
        xT_r = xT_d.rearrange("(t p) s -> p t s", p=P)
        nc.sync.dma_start(xT_sb[:, 0:1, :], xT_r[:, 0:1, :])
        wk_sb = persist.tile([P, H // P, OC], DT)
        nc.sync.dma_start(wk_sb[:], wkT_d.rearrange("(t p) c -> p t c", p=P))
        for kt in range(1, H // P):
            nc.sync.dma_start(xT_sb[:, kt:kt + 1, :], xT_r[:, kt:kt + 1, :])
        wv_sb = persist.tile([P, H // P, OC], DT)
        nc.sync.dma_start(wv_sb[:], wvT_d.rearrange("(t p) c -> p t c", p=P))
        wo_sb = persist.tile([P, OC // P, H], DT)         # [128, 2, 1024]
        nc.sync.dma_start(wo_sb[:], woT_d.rearrange("(t p) o -> p t o", p=P))

        QT_sb = persist.tile([P, OC // P, S], DT)         # [128, 2, 2048]
        KT_sb = persist.tile([P, OC // P, S], DT)
        V_sb = persist.tile([P, NKT, HPG * (HD + 1)], DT)  # [128, 16, 260]
        ctxT_sb = persist.tile([P, OC // P, S], DT)
        avg_slots = persist.tile([P, NKT, PAIRS * QQ], F32)
        avg_sb = persist.tile([P, NKT], F32)

        # ---- phase 1: Q/K/V projections ---------------------------------
        with tc.tile_pool(name="p1psum", bufs=2, space="PSUM") as p1:
            for which, w_sb, dst, bias in (
                ("q", wq_sb, QT_sb, True),
                ("k", wk_sb, KT_sb, False),
            ):
                for m in range(OC // P):
                    for nh in range(2):  # halves of S
                        ps = p1.tile([P, S // 2], F32, tag="psqk")
                        for kt in range(H // P):
                            for ncj in range(2):
                                nc.tensor.matmul(
                                    ps[:, ncj * 512:(ncj + 1) * 512],
                                    lhsT=w_sb[:, kt, m * P:(m + 1) * P],
                                    rhs=xT_sb[:, kt,
                                              nh * 1024 + ncj * 512:
                                              nh * 1024 + (ncj + 1) * 512],
                                    start=(kt == 0),
                                    stop=(kt == H // P - 1),
                                )
                        if bias:
                            nc.scalar.activation(
                                dst[:, m, nh * 1024:(nh + 1) * 1024],
                                ps[:],
                                Ident,
                                bias=bq_sb[:, m:m + 1],
                            )
                        else:
                            nc.scalar.activation(
                                dst[:, m, nh * 1024:(nh + 1) * 1024],
                                ps[:],
                                Copy,
                            )
            # V projection is interleaved into the first attention sweep;
            # only its ones columns are prefilled here (disjoint from the
            # per-head 64-wide slices the copies write).
            v4 = V_sb[:].rearrange("p t (g c) -> p t g c", c=HD + 1)
            nc.vector.memset(v4[:, :, :, HD:HD + 1], 1.0)

        # ---- phase 2: attention main loop -------------------------------
        ones_f32 = persist.tile([1, P], F32)
        nc.vector.memset(ones_f32[:], 1.0)

        p2stack = ExitStack()
        with tc.tile_pool(name="expp", bufs=2) as expp, \
             tc.tile_pool(name="rp", bufs=2) as rp, \
             tc.tile_pool(name="scr", bufs=4) as scr:
            p2 = p2stack.enter_context(
                tc.tile_pool(name="p2psum", bufs=2, space="PSUM"))
            p2c = p2stack.enter_context(
                tc.tile_pool(name="p2ctx", bufs=2, space="PSUM"))
            # psv lives only during sweep 0; p2r opens after it closes so
            # both fit in the remaining two PSUM banks
            psv_stack = ExitStack()
            psv_pool = psv_stack.enter_context(
                tc.tile_pool(name="psv", bufs=2, space="PSUM"))
            p2r = None
            for pair in range(PAIRS):
                for qq in range(QQ):
                    q0 = qq * QW
                    expT = expp.tile([P, NKT, 2 * QW], DT, tag="expT")
                    cps = [
                        p2c.tile([HD + 1, QW], F32, tag="cps", name=f"cps{i}")
                        for i in range(2)
                    ]

                    def vmm(kt):
                        for hl in range(2):
                            h = 2 * pair + hl
                            nc.tensor.matmul(
                                cps[hl][:],
                                lhsT=V_sb[:, kt,
                                          h * (HD + 1):(h + 1) * (HD + 1)],
                                rhs=expT[:, kt, hl * QW:(hl + 1) * QW],
                                start=(kt == 0),
                                stop=(kt == NKT - 1),
                            )

                    # scores+exp run one kt ahead of the attn@V matmuls so
                    # PE never waits on the exp of the tile it consumes.
                    # Sweep 0 additionally emits the V projection for key
                    # tile kt just before using it one iteration later.
                    for kt in range(NKT):
                        if pair == 0 and qq == 0:
                            psv = psv_pool.tile([P, OC], F32, tag="psv")
                            for hkt in range(H // P):
                                nc.tensor.matmul(
                                    psv[:],
                                    lhsT=xT_sb[:, hkt, kt * P:(kt + 1) * P],
                                    rhs=wv_sb[:, hkt, :],
                                    start=(hkt == 0),
                                    stop=(hkt == H // P - 1),
                                )
                            nc.vector.tensor_copy(
                                v4[:, kt, :, 0:HD],
                                psv[:].rearrange("p (g c) -> p g c", c=HD),
                            )
                        ps_s = p2.tile([P, 2 * QW], F32, tag="ps_s")
                        for hl in range(2):
                            lo = 64 * hl
                            nc.tensor.matmul(
                                ps_s[:, hl * QW:(hl + 1) * QW],
                                lhsT=KT_sb[lo:lo + 64, pair,
                                           kt * P:(kt + 1) * P],
                                rhs=QT_sb[lo:lo + 64, pair, q0:q0 + QW],
                                start=True,
                                stop=True,
                            )
                        nc.scalar.activation(expT[:, kt, :], ps_s[:], Exp)
                        if kt > 0:
                            vmm(kt - 1)
                    vmm(NKT - 1)
                    if pair == 0 and qq == 0:
                        psv_stack.close()
                        p2r = p2stack.enter_context(
                            tc.tile_pool(name="p2rr", bufs=2, space="PSUM"))

                    # denominators -> r -> replicate across partitions via a
                    # K=1 PE broadcast matmul (cheap, no DMA round trip).
                    # Both heads' chains interleaved so they overlap.
                    rrep = rp.tile([P, 2 * QW], DT, tag="rrep")
                    r32s, rpss = [], []
                    for hl in range(2):
                        r32 = scr.tile([1, QW], F32, tag="r32",
                                       name=f"r32_{hl}")
                        nc.vector.reciprocal(r32[:], cps[hl][HD:HD + 1, :])
                        r32s.append(r32)
                    for hl in range(2):
                        rps = p2r.tile([P, QW], F32, tag="rps",
                                       name=f"rps_{hl}")
                        nc.tensor.matmul(rps[:], lhsT=ones_f32[:],
                                         rhs=r32s[hl][:], start=True,
                                         stop=True)
                        rpss.append(rps)
                    for hl in range(2):
                        nc.scalar.copy(
                            rrep[:, hl * QW:(hl + 1) * QW], rpss[hl][:])
                    for hl in range(2):
                        # normalized ctx (bf16) while copying out of psum
                        nc.vector.tensor_tensor(
                            ctxT_sb[64 * hl:64 * hl + 64, pair, q0:q0 + QW],
                            cps[hl][0:HD, :],
                            rrep[64 * hl:64 * hl + 64,
                                 hl * QW:(hl + 1) * QW],
                            MUL,
                        )

                    # avg_attention partial sums: the final sweep's batch is
                    # deferred past the output projection so it overlaps
                    slot = pair * QQ + qq
                    if not (pair == PAIRS - 1 and qq == QQ - 1):
                        for kt in range(NKT):
                            tt_out = scr.tile([P, 2 * QW], DT, tag="tt_out")
                            nc.vector.affine_mul_reduce(
                                out=tt_out[:],
                                accum_out=avg_slots[:, kt, slot:slot + 1],
                                in0=expT[:, kt, :],
                                in1=rrep[:],
                                scale=1.0,
                                bias=0.0,
                            )
                    else:
                        last_expT, last_rrep, last_slot = expT, rrep, slot

            # ---- phase 3: output projection -----------------------------
            # psum->sbuf copies go on ScalarE so they don't queue behind
            # the remaining avg reductions on DVE
            p2stack.close()  # release phase-2 psum pools
            with tc.tile_pool(name="p3psum", bufs=2, space="PSUM") as p3, \
                 tc.tile_pool(name="p3out", bufs=2) as p3o:
                for st in range(NKT):
                    po = p3.tile([P, H], F32, tag="po")
                    for ot in range(OC // P):
                        for ncj in range(2):
                            nc.tensor.matmul(
                                po[:, ncj * 512:(ncj + 1) * 512],
                                lhsT=ctxT_sb[:, ot, st * P:(st + 1) * P],
                                rhs=wo_sb[:, ot, ncj * 512:(ncj + 1) * 512],
                                start=(ot == 0),
                                stop=(ot == OC // P - 1),
                            )
                    ost = p3o.tile([P, H], F32, tag="ost")
                    nc.scalar.copy(ost[:], po[:])
                    nc.sync.dma_start(out_d[st * P:(st + 1) * P, :], ost[:])
                    tt_out = scr.tile([P, 2 * QW], DT, tag="tt_out")
                    nc.vector.affine_mul_reduce(
                        out=tt_out[:],
                        accum_out=avg_slots[:, st, last_slot:last_slot + 1],
                        in0=last_expT[:, st, :],
                        in1=last_rrep[:],
                        scale=1.0,
                        bias=0.0,
                    )

        nc.vector.tensor_reduce(
            avg_sb[:], avg_slots[:], axis=mybir.AxisListType.X, op=ADD
        )
        nc.sync.dma_start(avg_d[:, :], avg_sb[:])

    nc.compile()
    return nc


def _get_nc():
    if "nc" not in _CACHE:
        _CACHE["nc"] = _trace_kernel()
    return _CACHE["nc"]


def _prep_in_maps(x, Wq, Wk, Wv, Wo, bq, bk, bv, bo):
    scale = HD ** -0.5
    in_maps = []
    for c in range(NCORES):
        b, g = divmod(c, GROUPS)
        sl = slice(g * OC, (g + 1) * OC)
        in_maps.append({
            "xT": np.ascontiguousarray(x[b].T).astype(NPDT),
            "wqT": np.ascontiguousarray((Wq[sl] * scale).T).astype(NPDT),
            "wkT": np.ascontiguousarray(Wk[sl].T).astype(NPDT),
            "wvT": np.ascontiguousarray(Wv[sl].T).astype(NPDT),
            "woT": np.ascontiguousarray(Wo[:, sl].T).astype(NPDT),
            "bq": (bq[sl] * scale).astype(np.float32),
        })
    return in_maps


def _postprocess(results, Wo, bv, bo):
    bo_eff = (bo + bv @ Wo.T).astype(np.float32)
    out = np.zeros((B, S, H), np.float32)
    avg = np.zeros((B, S), np.float32)
    for c in range(NCORES):
        b = c // GROUPS
        out[b] += results[c]["out_part"]
        avg[b] += results[c]["avg_part"].T.ravel()
    out += bo_eff
    avg /= NH * S
    return out, avg


def kernel(x, Wq, Wk, Wv, Wo, bq, bk, bv, bo, _trace=False):
    x, Wq, Wk, Wv, Wo = (np.asarray(a, np.float32) for a in (x, Wq, Wk, Wv, Wo))
    bq, bk, bv, bo = (np.asarray(a, np.float32) for a in (bq, bk, bv, bo))
    nc = _get_nc()
    in_maps = _prep_in_maps(x, Wq, Wk, Wv, Wo, bq, bk, bv, bo)
    res = run_bass_kernel_spmd(nc, in_maps, list(range(NCORES)), trace=_trace)
    out, avg = _postprocess(res.results, Wo, bv, bo)
    if _trace:
        _CACHE["last_result"] = res
    return out, avg
